# revision 5
# baseline (speedup 1.0000x reference)
"""NodeToEdge GNN message passing on 8 Trainium2 NeuronCores.

Reference computation (per edge e with endpoints a=edge_ids[0,e], b=edge_ids[1,e]):
    out[e] = (node_src_feats[a] + off_edge_src[e]) * (node_tgt_feats[b] + off_edge_tgt[e])

Strategy (edge-sharded, per spec hint):
  - 300000 edges padded to 303104 and split into 8 slabs of EC=37888.
  - Per core, the node tables are COMPACTED to only the rows that core's
    edges reference (~26.5K unique ids per table < 32768), so edge ids can
    be remapped to int16 positions and gathered with the fast SWDGE
    dma_gather instruction (one instruction per 1024 edges instead of one
    DMA descriptor-generation instruction per 128 edges).
  - Per super-block of NI=1024 edges: two dma_gathers (src/tgt rows),
    two streaming loads (offsets), three DVE elementwise ops, one store.
    The index lists are host-permuted so that gather output, offset loads
    and stores all use the natural partition-contiguous SBUF layout.
"""

import sys

if "/opt/trn_rl_repo" not in sys.path:
    sys.path.insert(0, "/opt/trn_rl_repo")

import numpy as np

import concourse.bacc as bacc
import concourse.bass as bass
import concourse.mybir as mybir
import concourse.tile as tile
from concourse.bass_utils import run_bass_kernel_spmd

NUM_NODES = 50000
FEAT = 256
NUM_EDGES = 300000

N_CORES = 8
NI = 1024                      # edges per super-block (one dma_gather pair)
K = NI // 128                  # gathered rows per partition per super-block
W = NI // 16                   # int16 idx columns per super-block per table
EC = 37888                     # edges per core (divisible by NI)
NSB = EC // NI                 # super-blocks per core
EPAD = EC * N_CORES            # 303104 >= NUM_EDGES
VCAP = 30000                   # compacted-table row capacity (unique ids ~26.6K)


def _build_bass():
    nc = bacc.Bacc(None, target_bir_lowering=False)
    tab_src = nc.dram_tensor("tab_src", [VCAP, FEAT], mybir.dt.float32, kind="ExternalInput")
    tab_tgt = nc.dram_tensor("tab_tgt", [VCAP, FEAT], mybir.dt.float32, kind="ExternalInput")
    idx_all = nc.dram_tensor("idx_all", [128, NSB * 2 * W], mybir.dt.int16, kind="ExternalInput")
    off_s = nc.dram_tensor("off_s", [EC, FEAT], mybir.dt.float32, kind="ExternalInput")
    off_t = nc.dram_tensor("off_t", [EC, FEAT], mybir.dt.float32, kind="ExternalInput")
    out = nc.dram_tensor("out", [EC, FEAT], mybir.dt.float32, kind="ExternalOutput")

    with tile.TileContext(nc) as tc:
        with tc.tile_pool(name="idxp", bufs=1) as idxp, \
             tc.tile_pool(name="offp", bufs=6) as offp, \
             tc.tile_pool(name="gp", bufs=8) as gp:
            idx_t = idxp.tile([128, NSB * 2 * W], mybir.dt.int16)
            nc.sync.dma_start(out=idx_t[:], in_=idx_all[:])
            for s in range(NSB):
                c0 = s * 2 * W
                gs = gp.tile([128, K, FEAT], mybir.dt.float32, tag="gs")
                gt = gp.tile([128, K, FEAT], mybir.dt.float32, tag="gt")
                os_ = offp.tile([128, K, FEAT], mybir.dt.float32, tag="os")
                ot_ = offp.tile([128, K, FEAT], mybir.dt.float32, tag="ot")
                nc.gpsimd.dma_gather(gs[:], tab_src[:], idx_t[:, c0:c0 + W], NI, NI, FEAT)
                nc.gpsimd.dma_gather(gt[:], tab_tgt[:], idx_t[:, c0 + W:c0 + 2 * W], NI, NI, FEAT)
                nc.scalar.dma_start(out=os_[:], in_=off_s[s * NI:(s + 1) * NI, :])
                nc.scalar.dma_start(out=ot_[:], in_=off_t[s * NI:(s + 1) * NI, :])
                nc.vector.tensor_tensor(out=gs[:], in0=gs[:], in1=os_[:], op=mybir.AluOpType.add)
                nc.vector.tensor_tensor(out=gt[:], in0=gt[:], in1=ot_[:], op=mybir.AluOpType.add)
                nc.vector.tensor_tensor(out=os_[:], in0=gs[:], in1=gt[:], op=mybir.AluOpType.mult)
                nc.sync.dma_start(out=out[s * NI:(s + 1) * NI, :], in_=os_[:])
    nc.compile()
    return nc


def _wrap_idx(inv):
    """[EC] compact ids -> [NSB, 128, W] int16 in dma_gather layout.

    Gather i-th index sits at wrapped[i%16, i//16] (replicated to 128
    partitions) and lands at out[i%128... specifically out[p, j] = row of
    idx position j*128+p. We want out[p, j] = edge s*NI + p*K + j, so
    idx position j*128+p must hold that edge's id."""
    a = inv.reshape(NSB, 128, K)                       # [s, p, j]
    lst = a.transpose(0, 2, 1).reshape(NSB, NI)        # [s, i], i = j*128+p
    w16 = lst.reshape(NSB, W, 16).transpose(0, 2, 1)   # [s, 16, W]
    return np.ascontiguousarray(np.tile(w16, (1, 8, 1)))


def _prepare_core(ids_s, ids_t, node_src, node_tgt):
    u_s, inv_s = np.unique(ids_s, return_inverse=True)
    u_t, inv_t = np.unique(ids_t, return_inverse=True)
    assert len(u_s) <= VCAP and len(u_t) <= VCAP, (len(u_s), len(u_t))
    tab_s = np.zeros((VCAP, FEAT), np.float32)
    tab_s[:len(u_s)] = node_src[u_s]
    tab_t = np.zeros((VCAP, FEAT), np.float32)
    tab_t[:len(u_t)] = node_tgt[u_t]
    ws = _wrap_idx(inv_s.astype(np.uint16))            # [NSB, 128, W]
    wt = _wrap_idx(inv_t.astype(np.uint16))
    idx_all = np.concatenate([ws, wt], axis=2)         # [NSB, 128, 2W]
    idx_all = np.ascontiguousarray(
        idx_all.transpose(1, 0, 2).reshape(128, NSB * 2 * W)).view(np.int16)
    return tab_s, tab_t, idx_all


def _run(inputs, trace=False, trace_kwargs=None):
    node_src = np.asarray(inputs["node_src_feats"], np.float32)
    node_tgt = np.asarray(inputs["node_tgt_feats"], np.float32)
    edge_ids = np.asarray(inputs["edge_ids"])
    off_s = np.asarray(inputs["off_edge_src"], np.float32)
    off_t = np.asarray(inputs["off_edge_tgt"], np.float32)

    ids_s = np.zeros(EPAD, np.int64)
    ids_t = np.zeros(EPAD, np.int64)
    ids_s[:NUM_EDGES] = edge_ids[0]
    ids_t[:NUM_EDGES] = edge_ids[1]
    offs_p = np.zeros((EPAD, FEAT), np.float32)
    offt_p = np.zeros((EPAD, FEAT), np.float32)
    offs_p[:NUM_EDGES] = off_s
    offt_p[:NUM_EDGES] = off_t

    in_maps = []
    for c in range(N_CORES):
        sl = slice(c * EC, (c + 1) * EC)
        tab_s, tab_t, idx_all = _prepare_core(ids_s[sl], ids_t[sl], node_src, node_tgt)
        in_maps.append({
            "tab_src": tab_s,
            "tab_tgt": tab_t,
            "idx_all": idx_all,
            "off_s": offs_p[sl],
            "off_t": offt_p[sl],
        })

    nc = _build_bass()
    kw = dict(trace_kwargs or {})
    res = run_bass_kernel_spmd(nc, in_maps, list(range(N_CORES)), trace=trace, **kw)
    out = np.concatenate([res.results[c]["out"] for c in range(N_CORES)], axis=0)
    return out[:NUM_EDGES], res


def kernel(**inputs) -> np.ndarray:
    out, _ = _run(inputs, trace=False)
    return out


# revision 7
# speedup vs baseline: 1.0117x; 1.0117x over previous
"""NodeToEdge GNN message passing on 8 Trainium2 NeuronCores.

Reference computation (per edge e with endpoints a=edge_ids[0,e], b=edge_ids[1,e]):
    out[e] = (node_src_feats[a] + off_edge_src[e]) * (node_tgt_feats[b] + off_edge_tgt[e])

Strategy (edge-sharded, per spec hint):
  - 300000 edges padded to 303104 and split into 8 slabs of EC=37888.
  - Per core, the node tables are COMPACTED to only the rows that core's
    edges reference (~26.5K unique ids per table < 32768), so edge ids can
    be remapped to int16 positions and gathered with the fast SWDGE
    dma_gather instruction (one instruction per 1024 edges instead of one
    DMA descriptor-generation instruction per 128 edges).
  - Per super-block of NI=1024 edges: two dma_gathers (src/tgt rows),
    two streaming loads (offsets), three DVE elementwise ops, one store.
    The index lists are host-permuted so that gather output, offset loads
    and stores all use the natural partition-contiguous SBUF layout.
"""

import sys

if "/opt/trn_rl_repo" not in sys.path:
    sys.path.insert(0, "/opt/trn_rl_repo")

import numpy as np

import concourse.bacc as bacc
import concourse.bass as bass
import concourse.mybir as mybir
import concourse.tile as tile
from concourse.bass_utils import run_bass_kernel_spmd

NUM_NODES = 50000
FEAT = 256
NUM_EDGES = 300000

N_CORES = 8
NI = 1024                      # edges per super-block (one dma_gather pair)
K = NI // 128                  # gathered rows per partition per super-block
W = NI // 16                   # int16 idx columns per super-block per table
EC = 37888                     # edges per core (divisible by NI)
NSB = EC // NI                 # super-blocks per core
EPAD = EC * N_CORES            # 303104 >= NUM_EDGES
VCAP = 30000                   # compacted-table row capacity (unique ids ~26.6K)


def _build_bass():
    nc = bacc.Bacc(None, target_bir_lowering=False, dynamic_dma_scratch_size=32768)
    tab_src = nc.dram_tensor("tab_src", [VCAP, FEAT], mybir.dt.float32, kind="ExternalInput")
    tab_tgt = nc.dram_tensor("tab_tgt", [VCAP, FEAT], mybir.dt.float32, kind="ExternalInput")
    idx_all = nc.dram_tensor("idx_all", [128, NSB * 2 * W], mybir.dt.int16, kind="ExternalInput")
    off_s = nc.dram_tensor("off_s", [EC, FEAT], mybir.dt.float32, kind="ExternalInput")
    off_t = nc.dram_tensor("off_t", [EC, FEAT], mybir.dt.float32, kind="ExternalInput")
    out = nc.dram_tensor("out", [EC, FEAT], mybir.dt.float32, kind="ExternalOutput")

    with tile.TileContext(nc) as tc:
        with tc.tile_pool(name="idxp", bufs=1) as idxp, \
             tc.tile_pool(name="offp", bufs=5) as offp, \
             tc.tile_pool(name="gp", bufs=8) as gp:
            idx_t = idxp.tile([128, NSB * 2 * W], mybir.dt.int16)
            nc.sync.dma_start(out=idx_t[:], in_=idx_all[:])
            for s in range(NSB):
                c0 = s * 2 * W
                gs = gp.tile([128, K, FEAT], mybir.dt.float32, tag="gs")
                gt = gp.tile([128, K, FEAT], mybir.dt.float32, tag="gt")
                os_ = offp.tile([128, K, FEAT], mybir.dt.float32, tag="os")
                ot_ = offp.tile([128, K, FEAT], mybir.dt.float32, tag="ot")
                nc.gpsimd.dma_gather(gs[:], tab_src[:], idx_t[:, c0:c0 + W], NI, NI, FEAT)
                nc.gpsimd.dma_gather(gt[:], tab_tgt[:], idx_t[:, c0 + W:c0 + 2 * W], NI, NI, FEAT)
                nc.sync.dma_start(out=os_[:], in_=off_s[s * NI:(s + 1) * NI, :])
                nc.sync.dma_start(out=ot_[:], in_=off_t[s * NI:(s + 1) * NI, :])
                nc.vector.tensor_tensor(out=gs[:], in0=gs[:], in1=os_[:], op=mybir.AluOpType.add)
                nc.vector.tensor_tensor(out=gt[:], in0=gt[:], in1=ot_[:], op=mybir.AluOpType.add)
                nc.vector.tensor_tensor(out=gs[:], in0=gs[:], in1=gt[:], op=mybir.AluOpType.mult)
                nc.sync.dma_start(out=out[s * NI:(s + 1) * NI, :], in_=gs[:])
    nc.compile()
    return nc


def _wrap_idx(inv):
    """[EC] compact ids -> [NSB, 128, W] int16 in dma_gather layout.

    Gather i-th index sits at wrapped[i%16, i//16] (replicated to 128
    partitions) and lands at out[i%128... specifically out[p, j] = row of
    idx position j*128+p. We want out[p, j] = edge s*NI + p*K + j, so
    idx position j*128+p must hold that edge's id."""
    a = inv.reshape(NSB, 128, K)                       # [s, p, j]
    lst = a.transpose(0, 2, 1).reshape(NSB, NI)        # [s, i], i = j*128+p
    w16 = lst.reshape(NSB, W, 16).transpose(0, 2, 1)   # [s, 16, W]
    return np.ascontiguousarray(np.tile(w16, (1, 8, 1)))


def _prepare_core(ids_s, ids_t, node_src, node_tgt):
    u_s, inv_s = np.unique(ids_s, return_inverse=True)
    u_t, inv_t = np.unique(ids_t, return_inverse=True)
    assert len(u_s) <= VCAP and len(u_t) <= VCAP, (len(u_s), len(u_t))
    tab_s = np.zeros((VCAP, FEAT), np.float32)
    tab_s[:len(u_s)] = node_src[u_s]
    tab_t = np.zeros((VCAP, FEAT), np.float32)
    tab_t[:len(u_t)] = node_tgt[u_t]
    ws = _wrap_idx(inv_s.astype(np.uint16))            # [NSB, 128, W]
    wt = _wrap_idx(inv_t.astype(np.uint16))
    idx_all = np.concatenate([ws, wt], axis=2)         # [NSB, 128, 2W]
    idx_all = np.ascontiguousarray(
        idx_all.transpose(1, 0, 2).reshape(128, NSB * 2 * W)).view(np.int16)
    return tab_s, tab_t, idx_all


def _run(inputs, trace=False, trace_kwargs=None):
    node_src = np.asarray(inputs["node_src_feats"], np.float32)
    node_tgt = np.asarray(inputs["node_tgt_feats"], np.float32)
    edge_ids = np.asarray(inputs["edge_ids"])
    off_s = np.asarray(inputs["off_edge_src"], np.float32)
    off_t = np.asarray(inputs["off_edge_tgt"], np.float32)

    ids_s = np.zeros(EPAD, np.int64)
    ids_t = np.zeros(EPAD, np.int64)
    ids_s[:NUM_EDGES] = edge_ids[0]
    ids_t[:NUM_EDGES] = edge_ids[1]
    offs_p = np.zeros((EPAD, FEAT), np.float32)
    offt_p = np.zeros((EPAD, FEAT), np.float32)
    offs_p[:NUM_EDGES] = off_s
    offt_p[:NUM_EDGES] = off_t

    in_maps = []
    for c in range(N_CORES):
        sl = slice(c * EC, (c + 1) * EC)
        tab_s, tab_t, idx_all = _prepare_core(ids_s[sl], ids_t[sl], node_src, node_tgt)
        in_maps.append({
            "tab_src": tab_s,
            "tab_tgt": tab_t,
            "idx_all": idx_all,
            "off_s": offs_p[sl],
            "off_t": offt_p[sl],
        })

    nc = _build_bass()
    kw = dict(trace_kwargs or {})
    res = run_bass_kernel_spmd(nc, in_maps, list(range(N_CORES)), trace=trace, **kw)
    out = np.concatenate([res.results[c]["out"] for c in range(N_CORES)], axis=0)
    return out[:NUM_EDGES], res


def kernel(**inputs) -> np.ndarray:
    out, _ = _run(inputs, trace=False)
    return out
